# revision 8
# baseline (speedup 1.0000x reference)
"""MultiHeadAttention forward on 8 Trainium2 NeuronCores.

Reference:  x:[2,2048,1024], fused QKV (W_qkv:[3072,1024]), 16 heads x d_k=64,
softmax(QK^T/8)V, output projection W_o:[1024,1024].

Sharding: core c handles batch b = c//4 and head group g = c%4 (heads
4g..4g+3, i.e. a 256-wide slice of the model dim).  Each core computes its
partial output-projection contribution out_partial = attn_slice @ W_o[:, sl].T
(shape [2048,1024], bf16); the host sums the 4 partials per batch in f32 and
adds b_o.

Device layouts (all host-prepped, transposed so that matmul contraction is
always the SBUF partition dim):
  xt      [1024, 2048]  = x[b].T
  wqkv_t  [1024,  768]  = per-core W_qkv rows, permuted [Q0..Q3|K0..K3|V0..V3].T
  wo_t    [ 256, 1024]  = W_o[:, 256g:256g+256].T
  b_qk    [ 128,    4]  column e = bias for e-block e (Q01,Q23,K01,K23)
  b_v     [ 128,  256]  v-bias broadcast across partitions
Output:
  out     [2048, 1024]  partial (pre-b_o) result for batch b, bf16

Schedule notes (the critical resource is ScalarE: 128 exp activations of
[128,1024] ~ 1.1us each = ~142us; everything else must hide under it):
  - input DMA is priority-ordered so the first Q01/K01 projection (and hence
    the first score matmul + exp) can start ~6us in, not ~30us.
  - 8 zero matmuls at t=0 warm the PE HAM clock gate during the DMA window.
  - softmax normalization uses reciprocal_approx_fast straight off the PSUM
    denominator row (no copy/DMA round trip).
  - output is written bf16 (halves the tail DMA); out-projection loops d2
    outermost so one LDWEIGHTS covers two matmuls.
"""

import sys

sys.path.insert(0, "/opt/trn_rl_repo")

import ml_dtypes
import numpy as np

import concourse.bass as bass
import concourse.mybir as mybir
import concourse.tile as tile
from concourse import bacc

F32 = mybir.dt.float32
BF16 = mybir.dt.bfloat16

D_MODEL = 1024
N_HEADS = 16
D_K = 64
B = 2
S = 2048
N_CORES = 8
HL = 4  # heads per core
D_SLICE = HL * D_K  # 256


def build_kernel():
    nc = bacc.Bacc("TRN2")

    xt = nc.dram_tensor("xt", [D_MODEL, S], BF16, kind="ExternalInput")
    wqkv_t = nc.dram_tensor("wqkv_t", [D_MODEL, 3 * D_SLICE], BF16, kind="ExternalInput")
    wo_t = nc.dram_tensor("wo_t", [D_SLICE, D_MODEL], BF16, kind="ExternalInput")
    b_qk = nc.dram_tensor("b_qk", [128, 4], F32, kind="ExternalInput")
    b_v = nc.dram_tensor("b_v", [128, D_SLICE], F32, kind="ExternalInput")
    out = nc.dram_tensor("out", [S, D_MODEL], BF16, kind="ExternalOutput")

    DC = D_MODEL // 128  # 8 contraction chunks for the QKV projection
    NT128 = S // 128  # 16

    with tile.TileContext(nc) as tc:
        with tc.tile_pool(name="persist", bufs=1) as pp:
            # ---- persistent SBUF tensors ----
            xt_sb = [pp.tile([128, S], BF16, name=f"xt{i}", tag=f"xt{i}") for i in range(DC)]
            wq_sb = [pp.tile([128, 3 * D_SLICE], BF16, name=f"wq{i}", tag=f"wq{i}") for i in range(DC)]
            wo_sb = [pp.tile([128, D_MODEL], BF16, name=f"wo{i}", tag=f"wo{i}") for i in range(2)]
            bqk_sb = pp.tile([128, 4], F32, name="bqk", tag="bqk")
            bv_sb = pp.tile([128, D_SLICE], F32, name="bv", tag="bv")
            ones_sb = pp.tile([128, 1], F32, name="ones", tag="ones")
            zw_sb = pp.tile([128, 512], BF16, name="zw", tag="zw")
            # qk_sb[0]=Q heads01, [1]=Q heads23, [2]=K heads01, [3]=K heads23
            qk_sb = [pp.tile([128, S], BF16, name=f"qk{i}", tag=f"qk{i}") for i in range(4)]
            # v_sb[j]: seq tile j, 4 head blocks of 65 cols: [V_h (64) | ones]
            v_sb = [pp.tile([128, HL * 65], BF16, name=f"v{j}", tag=f"v{j}") for j in range(NT128)]
            # ot_sb[hp]: attention output^T, heads (2hp, 2hp+1) stacked
            ot_sb = [pp.tile([128, S], BF16, name=f"ot{i}", tag=f"ot{i}") for i in range(2)]

            with tc.tile_pool(name="psum", bufs=2, space="PSUM") as ps_pool, \
                 tc.tile_pool(name="work", bufs=2) as wk_pool, \
                 tc.tile_pool(name="den", bufs=2) as dn_pool:

                nc.vector.memset(zw_sb[:], 0.0)
                nc.vector.memset(ones_sb[:], 1.0)

                # ---- priority-ordered input DMAs ----
                nc.sync.dma_start(bqk_sb[:], b_qk[:])
                nc.sync.dma_start(bv_sb[:], b_v[:])
                # 1) Q01 + K01 weight slices and the first x column block:
                #    everything the first scores+exp needs.
                for i in range(DC):
                    rs = slice(128 * i, 128 * (i + 1))
                    nc.sync.dma_start(wq_sb[i][:, 0:128], wqkv_t[rs, 0:128])
                    nc.sync.dma_start(wq_sb[i][:, 256:384], wqkv_t[rs, 256:384])
                    nc.sync.dma_start(xt_sb[i][:, 0:512], xt[rs, 0:512])
                # 2) Q23 + K23 weight slices
                for i in range(DC):
                    rs = slice(128 * i, 128 * (i + 1))
                    nc.sync.dma_start(wq_sb[i][:, 128:256], wqkv_t[rs, 128:256])
                    nc.sync.dma_start(wq_sb[i][:, 384:512], wqkv_t[rs, 384:512])
                # 3) V weights
                for i in range(DC):
                    rs = slice(128 * i, 128 * (i + 1))
                    nc.sync.dma_start(wq_sb[i][:, 512:768], wqkv_t[rs, 512:768])
                # 4) output-projection weights (needed last)
                for i in range(2):
                    nc.sync.dma_start(wo_sb[i][:], wo_t[128 * i : 128 * (i + 1), :])

                # ====== Phase-1 helpers ======
                def dma_x_block(cb):
                    cs = slice(512 * cb, 512 * (cb + 1))
                    for dc in range(DC):
                        nc.sync.dma_start(xt_sb[dc][:, cs], xt[128 * dc : 128 * (dc + 1), cs])

                def proj_qk(cb, ebs):
                    """Project Q/K e-blocks `ebs` for x column block cb."""
                    cs = slice(512 * cb, 512 * (cb + 1))
                    for eb in ebs:
                        ps = ps_pool.tile([128, 512], F32, name="pq", tag="pj", bufs=2)
                        for dc in range(DC):
                            nc.tensor.matmul(
                                ps[:],
                                wq_sb[dc][:, 128 * eb : 128 * (eb + 1)],
                                xt_sb[dc][:, cs],
                                start=(dc == 0),
                                stop=(dc == DC - 1),
                            )
                        with nc.allow_low_precision(reason="bf16 activations"):
                            nc.vector.tensor_scalar_add(
                                qk_sb[eb][:, cs], in0=ps[:], scalar1=bqk_sb[:, eb : eb + 1]
                            )

                def proj_v(cb):
                    """Project V for the 4 seq chunks of x column block cb."""
                    for jj in range(4):
                        j = 4 * cb + jj
                        ps = ps_pool.tile([128, 512], F32, name="pv", tag="pj", bufs=2)
                        psv = ps[:, 0:D_SLICE]
                        for dc in range(DC):
                            nc.tensor.matmul(
                                psv,
                                xt_sb[dc][:, 128 * j : 128 * (j + 1)],
                                wq_sb[dc][:, 2 * D_SLICE : 3 * D_SLICE],
                                start=(dc == 0),
                                stop=(dc == DC - 1),
                            )
                        vt = v_sb[j][:].rearrange("p (g x) -> p g x", x=65)
                        nc.vector.tensor_copy(
                            vt[:, :, 64:65],
                            ones_sb[:][:, None, :].broadcast_to((128, HL, 1)),
                        )
                        p3 = psv.rearrange("p (g x) -> p g x", x=64)
                        b3 = bv_sb[:].rearrange("p (g x) -> p g x", x=64)
                        with nc.allow_low_precision(reason="bf16 activations"):
                            nc.vector.tensor_add(vt[:, :, 0:64], p3, b3)

                # ====== Phase-2 helpers ======
                def attention_begin(st, hp):
                    # pva/pvb are allocated lazily at the first attention_pvs
                    # call: a pool tile's allocation point anchors its users'
                    # scheduling priority, and the PV matmuls must not be
                    # hoisted ahead of the V projection they consume.
                    return {"st": st, "hp": hp}

                def attention_alloc_pv(a):
                    if "pva" not in a:
                        a["pva"] = ps_pool.tile([65, 512], F32, name="pva", tag="pva", bufs=1)
                        a["pvb"] = ps_pool.tile([65, 512], F32, name="pvb", tag="pvb", bufs=1)

                def attention_scores(a, kcs):
                    st, hp = a["st"], a["hp"]
                    qs = slice(512 * st, 512 * (st + 1))
                    q_t, k_t = qk_sb[hp], qk_sb[2 + hp]
                    for kc in kcs:
                        ks = slice(128 * kc, 128 * (kc + 1))
                        sc = ps_pool.tile([128, 1024], F32, name="sc", tag="sc", bufs=2)
                        nc.tensor.matmul(
                            sc[:, 0:512], k_t[0:64, ks], q_t[0:64, qs],
                            start=True, stop=True, tile_position=(0, 0),
                            skip_group_check=True,
                        )
                        nc.tensor.matmul(
                            sc[:, 512:1024], k_t[64:128, ks], q_t[64:128, qs],
                            start=True, stop=True, tile_position=(64, 0),
                            skip_group_check=True,
                        )
                        eab = wk_pool.tile([128, 1024], BF16, name="eab", tag="eab", bufs=44)
                        nc.scalar.activation(
                            eab[:], sc[:], mybir.ActivationFunctionType.Exp,
                            scale=0.125,
                        )
                        a.setdefault("eabs", {})[kc] = eab

                def attention_pvs(a, kcs):
                    attention_alloc_pv(a)
                    hp = a["hp"]
                    for kc in kcs:
                        eab = a["eabs"].pop(kc)
                        for ph, pv_ps in ((0, a["pva"]), (1, a["pvb"])):
                            h = 2 * hp + ph
                            nc.tensor.matmul(
                                pv_ps[:],
                                v_sb[kc][:, 65 * h : 65 * h + 65],
                                eab[:, 512 * ph : 512 * (ph + 1)],
                                start=(kc == 0),
                                stop=(kc == NT128 - 1),
                                skip_group_check=True,
                            )

                def attention_kcs(a, kcs):
                    kcs = list(kcs)
                    attention_scores(a, kcs)
                    attention_pvs(a, kcs)

                def attention_finish(a):
                    st, hp = a["st"], a["hp"]
                    qs = slice(512 * st, 512 * (st + 1))
                    for ph, pv_ps in ((0, a["pva"]), (1, a["pvb"])):
                        dr = dn_pool.tile([1, 512], F32, name="dr", tag="dr", bufs=2)
                        nc.vector.reciprocal(dr[0:1, :], pv_ps[64:65, :])
                        bc = dn_pool.tile([64, 512], F32, name="bc", tag="bc", bufs=2)
                        nc.gpsimd.partition_broadcast(bc[:], dr[0:1, :])
                        with nc.allow_low_precision(reason="bf16 activations"):
                            nc.vector.tensor_mul(
                                ot_sb[hp][64 * ph : 64 * (ph + 1), qs],
                                pv_ps[0:64, :],
                                bc[:],
                            )

                def outproj_block(st):
                    for jj in range(4):
                        j = 4 * st + jj
                        js = slice(128 * j, 128 * (j + 1))
                        pos = [
                            ps_pool.tile([128, 512], F32, name="po", tag="pj", bufs=2)
                            for _ in range(2)
                        ]
                        for d2 in range(2):
                            for nb in range(2):
                                ns = slice(512 * nb, 512 * (nb + 1))
                                nc.tensor.matmul(
                                    pos[nb][:],
                                    ot_sb[d2][:, js],
                                    wo_sb[d2][:, ns],
                                    start=(d2 == 0),
                                    stop=(d2 == 1),
                                )
                        for nb in range(2):
                            ns = slice(512 * nb, 512 * (nb + 1))
                            ob = wk_pool.tile([128, 512], BF16, name="ob", tag="ob", bufs=2)
                            with nc.allow_low_precision(reason="bf16 output"):
                                nc.vector.tensor_copy(ob[:], pos[nb][:])
                            nc.sync.dma_start(out[js, ns], ob[:])

                # ====== interleaved emission: proj blocks feed attention ====
                proj_qk(0, (0, 2))
                a00 = attention_begin(0, 0)
                attention_scores(a00, range(0, 4))
                proj_qk(0, (1, 3))
                a01 = attention_begin(0, 1)
                attention_scores(a01, range(0, 4))
                proj_v(0)
                attention_pvs(a00, range(0, 4))
                attention_pvs(a01, range(0, 4))
                dma_x_block(1)
                proj_qk(1, (0, 2, 1, 3))
                proj_v(1)
                attention_kcs(a00, range(4, 8))
                attention_kcs(a01, range(4, 8))
                a10 = attention_begin(1, 0)
                a11 = attention_begin(1, 1)
                attention_scores(a10, range(0, 4))
                attention_scores(a11, range(0, 4))
                dma_x_block(2)
                proj_qk(2, (0, 2, 1, 3))
                proj_v(2)
                attention_kcs(a00, range(8, 12))
                attention_kcs(a01, range(8, 12))
                attention_scores(a10, range(4, 8))
                attention_scores(a11, range(4, 8))
                dma_x_block(3)
                proj_qk(3, (0, 2, 1, 3))
                proj_v(3)
                attention_kcs(a00, range(12, 16))
                attention_kcs(a01, range(12, 16))
                attention_finish(a00)
                attention_finish(a01)
                attention_scores(a10, range(8, 16))
                attention_scores(a11, range(8, 16))
                a20 = attention_begin(2, 0)
                a21 = attention_begin(2, 1)
                attention_scores(a20, range(0, 4))
                attention_scores(a21, range(0, 4))
                attention_pvs(a10, range(0, 16))
                attention_finish(a10)
                attention_pvs(a11, range(0, 16))
                attention_finish(a11)
                outproj_block(0)
                attention_scores(a20, range(4, 16))
                attention_scores(a21, range(4, 16))
                a30 = attention_begin(3, 0)
                a31 = attention_begin(3, 1)
                attention_scores(a30, range(0, 4))
                attention_scores(a31, range(0, 4))
                attention_pvs(a20, range(0, 16))
                attention_finish(a20)
                attention_pvs(a21, range(0, 16))
                attention_finish(a21)
                outproj_block(1)
                attention_scores(a30, range(4, 16))
                attention_pvs(a30, range(0, 16))
                attention_finish(a30)
                attention_scores(a31, range(4, 16))
                attention_pvs(a31, range(0, 16))
                attention_finish(a31)
                outproj_block(2)
                outproj_block(3)

    nc.compile()
    return nc


def make_in_maps(x, W_qkv, b_qkv, W_o):
    """Per-core input dicts (host-side sharding + layout prep)."""
    x = np.asarray(x, np.float32)
    W_qkv = np.asarray(W_qkv, np.float32)
    b_qkv = np.asarray(b_qkv, np.float32)
    W_o = np.asarray(W_o, np.float32)

    in_maps = []
    xts = [np.ascontiguousarray(x[b].T).astype(ml_dtypes.bfloat16) for b in range(B)]
    for c in range(N_CORES):
        b, g = c // 4, c % 4
        heads = range(4 * g, 4 * g + 4)
        wq = [W_qkv[192 * h : 192 * h + 64] for h in heads]
        wk = [W_qkv[192 * h + 64 : 192 * h + 128] for h in heads]
        wv = [W_qkv[192 * h + 128 : 192 * h + 192] for h in heads]
        w_perm = np.concatenate(wq + wk + wv, axis=0)  # [768, 1024]
        bq = [b_qkv[192 * h : 192 * h + 64] for h in heads]
        bk = [b_qkv[192 * h + 64 : 192 * h + 128] for h in heads]
        bv = [b_qkv[192 * h + 128 : 192 * h + 192] for h in heads]
        b_perm = np.concatenate(bq + bk + bv)  # [768]
        in_maps.append(
            {
                "xt": xts[b],
                "wqkv_t": np.ascontiguousarray(w_perm.T).astype(ml_dtypes.bfloat16),
                "wo_t": np.ascontiguousarray(
                    W_o[:, 256 * g : 256 * g + 256].T
                ).astype(ml_dtypes.bfloat16),
                "b_qk": np.ascontiguousarray(b_perm[:512].reshape(4, 128).T),
                "b_v": np.ascontiguousarray(
                    np.broadcast_to(b_perm[512:], (128, 256))
                ),
            }
        )
    return in_maps


_NC = None


def kernel(x, W_qkv, b_qkv, W_o, b_o):
    global _NC
    from concourse.bass_utils import run_bass_kernel_spmd

    if _NC is None:
        _NC = build_kernel()
    in_maps = make_in_maps(x, W_qkv, b_qkv, W_o)
    res = run_bass_kernel_spmd(_NC, in_maps, core_ids=list(range(N_CORES)))
    b_o = np.asarray(b_o, np.float32)
    outs = [np.asarray(r["out"], np.float32) for r in res.results]
    full = np.empty((B, S, D_MODEL), np.float32)
    for b in range(B):
        full[b] = outs[4 * b] + outs[4 * b + 1] + outs[4 * b + 2] + outs[4 * b + 3]
        full[b] += b_o
    return full


# revision 42
# speedup vs baseline: 1.3596x; 1.3596x over previous
"""MultiHeadAttention forward on 8 Trainium2 NeuronCores.

Reference:  x:[2,2048,1024], fused QKV (W_qkv:[3072,1024]), 16 heads x d_k=64,
softmax(QK^T/8)V, output projection W_o:[1024,1024].

Sharding: core c handles batch b = c//4 and head group g = c%4 (heads
4g..4g+3, i.e. a 256-wide slice of the model dim).  Each core computes its
partial output-projection contribution out_partial = attn_slice @ W_o[:, sl].T
(shape [2048,1024], bf16); the host sums the 4 partials per batch in f32 and
adds b_o.

Device layouts (host-prepped; contraction is always the SBUF partition dim).
Weights are pre-split into three dram tensors so each loads as ONE large DMA
(small strided DMAs are descriptor-dominated: 32KB ~ 138GB/s vs 1MB ~ 341GB/s):
  xt      [1024, 2048]  = x[b].T                  -> xt_all  [128,8,2048]
  w_a     [1024,  256]  = [Q01|K01] rows .T       -> wq_all[:,:,  0:256]
  w_b     [1024,  256]  = [Q23|K23] rows .T       -> wq_all[:,:,256:512]
  w_v     [1024,  256]  = [V0..V3] rows .T        -> wq_all[:,:,512:768]
  wo_t    [ 256, 1024]  = W_o[:, 256g:256g+256].T -> wo_all [128,2,1024]
  b_qk    [ 128,    4]  column e = bias for e-block e (Q01,Q23,K01,K23)
  b_v     [ 128,  256]  v-bias broadcast across partitions
Output:
  out     [2048, 1024]  partial (pre-b_o) result for batch b, bf16; each
                        128-row block ships as one contiguous 256KB DMA

Schedule: ScalarE is the binding engine (128 exp activations of [128,1024],
~1.11us each ~ 142us).  The emission round-robins score matmuls (which feed
ScalarE) with the PV accumulations of the PREVIOUS head-pair block so the PE
never runs a long PV burst that starves ScalarE.  PSUM: 2 proj banks + 4
score banks (2 tiles x 2 bufs) + pva + pvb = 8.  A block's pva/pvb are
allocated lazily at its first PV emission -- allocating earlier lets the
scheduler hoist PV matmuls ahead of their producers, which deadlocks.
"""

import sys

sys.path.insert(0, "/opt/trn_rl_repo")

import ml_dtypes
import numpy as np

import concourse.bass as bass
import concourse.mybir as mybir
import concourse.tile as tile
from concourse import bacc

F32 = mybir.dt.float32
BF16 = mybir.dt.bfloat16

D_MODEL = 1024
N_HEADS = 16
D_K = 64
B = 2
S = 2048
N_CORES = 8
HL = 4  # heads per core
D_SLICE = HL * D_K  # 256

# e-block -> column offset in wq_all (layout Q01|K01|Q23|K23|V)
EBCOL = {0: 0, 2: 128, 1: 256, 3: 384}
DC = D_MODEL // 128  # 8 contraction chunks for the QKV projection


def build_kernel():
    nc = bacc.Bacc("TRN2")

    # inputs are host-permuted to SBUF layout: every DMA row is contiguous
    xts = [
        nc.dram_tensor(f"xt{cb}", [128, DC * 512], BF16, kind="ExternalInput")
        for cb in range(4)
    ]
    w_a1 = nc.dram_tensor("w_a1", [128, DC * 128], BF16, kind="ExternalInput")
    w_a2 = nc.dram_tensor("w_a2", [128, DC * 128], BF16, kind="ExternalInput")
    w_b = nc.dram_tensor("w_b", [128, DC * 256], BF16, kind="ExternalInput")
    w_v = nc.dram_tensor("w_v", [128, DC * 256], BF16, kind="ExternalInput")
    wo_t = nc.dram_tensor("wo_t", [128, 2 * D_MODEL], BF16, kind="ExternalInput")
    b_qk = nc.dram_tensor("b_qk", [128, 4], F32, kind="ExternalInput")
    b_v = nc.dram_tensor("b_v", [128, D_SLICE], F32, kind="ExternalInput")
    out = nc.dram_tensor("out", [S, D_MODEL], BF16, kind="ExternalOutput")

    NT128 = S // 128  # 16

    with tile.TileContext(nc) as tc:
        with tc.tile_pool(name="persist", bufs=1) as pp:
            # ---- persistent SBUF tensors ----
            xt_all = pp.tile([128, DC, S], BF16, name="xt_all", tag="xt_all")
            wq_all = pp.tile([128, DC, 3 * D_SLICE], BF16, name="wq_all", tag="wq_all")
            wo_all = pp.tile([128, 2, D_MODEL], BF16, name="wo_all", tag="wo_all")
            bqk_sb = pp.tile([128, 4], F32, name="bqk", tag="bqk")
            bv_sb = pp.tile([128, D_SLICE], F32, name="bv", tag="bv")
            ones_sb = pp.tile([128, 1], F32, name="ones", tag="ones")
            zw_sb = pp.tile([128, 512], BF16, name="zw", tag="zw")
            # qk_sb[0]=Q heads01, [1]=Q heads23, [2]=K heads01, [3]=K heads23
            qk_sb = [pp.tile([128, S], BF16, name=f"qk{i}", tag=f"qk{i}") for i in range(4)]
            # v_sb[j]: seq tile j, 4 head blocks of 65 cols: [V_h (64) | ones]
            v_sb = [pp.tile([128, HL * 65], BF16, name=f"v{j}", tag=f"v{j}") for j in range(NT128)]
            # ot_sb[hp]: attention output^T, heads (2hp, 2hp+1) stacked
            ot_sb = [pp.tile([128, S], BF16, name=f"ot{i}", tag=f"ot{i}") for i in range(2)]

            with tc.tile_pool(name="psum", bufs=2, space="PSUM") as ps_pool, \
                 tc.tile_pool(name="work", bufs=2) as wk_pool, \
                 tc.tile_pool(name="den", bufs=2) as dn_pool:

                # ---- PE warm-up: zero matmuls keep the HAM clock gate busy
                # while the first input DMAs land (cold PE runs at 1.2GHz) ----
                nc.vector.memset(zw_sb[:], 0.0)
                nc.vector.memset(ones_sb[:], 1.0)
                for w in range(8):
                    wps = ps_pool.tile([128, 512], F32, name="wps", tag="pj", bufs=2)
                    nc.tensor.matmul(
                        wps[:], zw_sb[:, 0:128], zw_sb[:], start=True, stop=True
                    )

                # ---- input DMAs: few, large, priority-ordered ----

                def dma_w(dram, cols, ncols):
                    nc.sync.dma_start(
                        wq_all[:, :, cols],
                        dram[:].rearrange("p (dc c) -> p dc c", c=ncols),
                    )

                def dma_x_block(cb):
                    cs = slice(512 * cb, 512 * (cb + 1))
                    nc.sync.dma_start(
                        xt_all[:, :, cs],
                        xts[cb][:].rearrange("p (dc c) -> p dc c", c=512),
                    )

                # single HWDGE queue drains FIFO, so issue order = priority;
                # the first pieces are split fine so the first projection
                # chain starts while the rest still streams
                dma_w(w_a1, slice(0, 128), 128)
                nc.sync.dma_start(
                    xt_all[:, 0:4, 0:512],
                    xts[0][:, 0:2048].rearrange("p (dc c) -> p dc c", c=512),
                )
                nc.sync.dma_start(
                    xt_all[:, 4:8, 0:512],
                    xts[0][:, 2048:4096].rearrange("p (dc c) -> p dc c", c=512),
                )
                dma_w(w_a2, slice(128, 256), 128)
                nc.sync.dma_start(bqk_sb[:], b_qk[:])
                nc.sync.dma_start(bv_sb[:], b_v[:])
                dma_w(w_b, slice(256, 512), 256)
                dma_x_block(1)
                dma_w(w_v, slice(512, 768), 256)
                dma_x_block(2)
                nc.sync.dma_start(
                    wo_all[:],
                    wo_t[:].rearrange("p (d c) -> p d c", c=D_MODEL),
                )
                dma_x_block(3)

                # ====== Phase-1 helpers (single accumulation chains) ======
                def proj_qk_chain(cb, eb):
                    cs = slice(512 * cb, 512 * (cb + 1))
                    col = EBCOL[eb]
                    ps = ps_pool.tile([128, 512], F32, name="pq", tag="pj", bufs=2)
                    for dc in range(DC):
                        nc.tensor.matmul(
                            ps[:],
                            wq_all[:, dc, col : col + 128],
                            xt_all[:, dc, cs],
                            start=(dc == 0),
                            stop=(dc == DC - 1),
                        )
                    with nc.allow_low_precision(reason="bf16 activations"):
                        nc.vector.tensor_scalar_add(
                            qk_sb[eb][:, cs], in0=ps[:], scalar1=bqk_sb[:, eb : eb + 1]
                        )

                def proj_v_chain(j):
                    ps = ps_pool.tile([128, 512], F32, name="pv", tag="pj", bufs=2)
                    psv = ps[:, 0:D_SLICE]
                    for dc in range(DC):
                        nc.tensor.matmul(
                            psv,
                            xt_all[:, dc, 128 * j : 128 * (j + 1)],
                            wq_all[:, dc, 512:768],
                            start=(dc == 0),
                            stop=(dc == DC - 1),
                        )
                    vt = v_sb[j][:].rearrange("p (g x) -> p g x", x=65)
                    nc.vector.tensor_copy(
                        vt[:, :, 64:65],
                        ones_sb[:][:, None, :].broadcast_to((128, HL, 1)),
                    )
                    p3 = psv.rearrange("p (g x) -> p g x", x=64)
                    b3 = bv_sb[:].rearrange("p (g x) -> p g x", x=64)
                    with nc.allow_low_precision(reason="bf16 activations"):
                        nc.vector.tensor_add(vt[:, :, 0:64], p3, b3)

                # ====== Phase-2 helpers ======
                blocks = [{"st": st, "hp": hp, "eabs": {}} for st in range(4) for hp in (0, 1)]

                def em_s(i, kcs):
                    """Scores + exp for block i, k-chunks kcs."""
                    a = blocks[i]
                    st, hp = a["st"], a["hp"]
                    qs = slice(512 * st, 512 * (st + 1))
                    q_t, k_t = qk_sb[hp], qk_sb[2 + hp]
                    for kc in kcs:
                        ks = slice(128 * kc, 128 * (kc + 1))
                        sc = ps_pool.tile([128, 1024], F32, name="sc", tag="sc", bufs=2)
                        nc.tensor.matmul(
                            sc[:, 0:512], k_t[0:64, ks], q_t[0:64, qs],
                            start=True, stop=True, tile_position=(0, 0),
                            skip_group_check=True,
                        )
                        nc.tensor.matmul(
                            sc[:, 512:1024], k_t[64:128, ks], q_t[64:128, qs],
                            start=True, stop=True, tile_position=(64, 0),
                            skip_group_check=True,
                        )
                        eab = wk_pool.tile([128, 1024], BF16, name="eab", tag="eab", bufs=44)
                        nc.scalar.activation(
                            eab[:], sc[:], mybir.ActivationFunctionType.Exp,
                            scale=0.125,
                        )
                        a["eabs"][kc] = eab

                def alloc_pv(i):
                    a = blocks[i]
                    if "pva" not in a:
                        tga, tgb, bufs = a.get("ptags", ("pva", "pvb", 1))
                        a["pva"] = ps_pool.tile([65, 512], F32, name="pva", tag=tga, bufs=bufs)
                        a["pvb"] = ps_pool.tile([65, 512], F32, name="pvb", tag=tgb, bufs=bufs)

                def em_p(i, kcs):
                    """PV accumulation for block i (lazy pva/pvb alloc)."""
                    a = blocks[i]
                    if "pva" not in a:
                        # the last block borrows the (idle by then) proj banks
                        # so its PV runs concurrently with block 6's finish
                        tga, tgb, bufs = a.get("ptags", ("pva", "pvb", 1))
                        a["pva"] = ps_pool.tile([65, 512], F32, name="pva", tag=tga, bufs=bufs)
                        a["pvb"] = ps_pool.tile([65, 512], F32, name="pvb", tag=tgb, bufs=bufs)
                    hp = a["hp"]
                    for kc in kcs:
                        eab = a["eabs"].pop(kc)
                        for ph, pv_ps in ((0, a["pva"]), (1, a["pvb"])):
                            h = 2 * hp + ph
                            nc.tensor.matmul(
                                pv_ps[:],
                                v_sb[kc][:, 65 * h : 65 * h + 65],
                                eab[:, 512 * ph : 512 * (ph + 1)],
                                start=(kc == 0),
                                stop=(kc == NT128 - 1),
                                skip_group_check=True,
                            )

                def em_f(i):
                    """Normalize block i: 1/denominator, broadcast, scale."""
                    a = blocks[i]
                    st, hp = a["st"], a["hp"]
                    qs = slice(512 * st, 512 * (st + 1))
                    # evacuate both PSUM banks FIRST (two fast copies) so the
                    # next block's PV chain gets the banks in ~1.4us; the slow
                    # normalize chain (reciprocal ~3.3us + broadcast + scale)
                    # then runs from the SBUF copies off the critical path
                    uns = []
                    for ph, pv_ps in ((0, a["pva"]), (1, a["pvb"])):
                        un = dn_pool.tile([65, 512], F32, name="un", tag="un", bufs=4)
                        nc.vector.tensor_copy(un[0:65, :], pv_ps[:])
                        uns.append(un)
                    # DVE reciprocal time scales with the free dim, so a
                    # [1,512] row costs 3.3us.  Scatter the 1024 denominators
                    # of this block across 128 partitions via DMA, take the
                    # reciprocal in ~0.2us, and gather back.  The scatter and
                    # gather use the same AP pair, so the (arbitrary) element
                    # mapping cancels.
                    dr2 = dn_pool.tile([128, 8], F32, name="dr2", tag="dr2", bufs=2)
                    nc.sync.dma_start(dr2[:, 0:4], uns[0][64:65, :])
                    nc.sync.dma_start(dr2[:, 4:8], uns[1][64:65, :])
                    rc2 = dn_pool.tile([128, 8], F32, name="rc2", tag="rc2", bufs=2)
                    nc.vector.reciprocal(rc2[:], dr2[:])
                    rcw = dn_pool.tile([1, 1024], F32, name="rcw", tag="rcw", bufs=2)
                    nc.sync.dma_start(rcw[0:1, 0:512], rc2[:, 0:4])
                    nc.sync.dma_start(rcw[0:1, 512:1024], rc2[:, 4:8])
                    for ph, un in ((0, uns[0]), (1, uns[1])):
                        bc = dn_pool.tile([64, 512], F32, name="bc", tag="bc", bufs=2)
                        nc.gpsimd.partition_broadcast(
                            bc[:], rcw[0:1, 512 * ph : 512 * (ph + 1)]
                        )
                        with nc.allow_low_precision(reason="bf16 activations"):
                            nc.vector.tensor_mul(
                                ot_sb[hp][64 * ph : 64 * (ph + 1), qs],
                                un[0:64, :],
                                bc[:],
                            )

                def outproj_j(j, tags=("pj", "pj"), tbufs=2):
                    if True:
                        js = slice(128 * j, 128 * (j + 1))
                        ob = wk_pool.tile([128, D_MODEL], BF16, name="ob", tag="ob", bufs=2)
                        pos = [
                            ps_pool.tile([128, 512], F32, name="po", tag=tags[k], bufs=tbufs)
                            for k in range(2)
                        ]
                        for d2 in range(2):
                            for nb in range(2):
                                nc.tensor.matmul(
                                    pos[nb][:],
                                    ot_sb[d2][:, js],
                                    wo_all[:, d2, 512 * nb : 512 * (nb + 1)],
                                    start=(d2 == 0),
                                    stop=(d2 == 1),
                                )
                        for nb in range(2):
                            with nc.allow_low_precision(reason="bf16 output"):
                                nc.vector.tensor_copy(
                                    ob[:, 512 * nb : 512 * (nb + 1)], pos[nb][:]
                                )
                        # out rows 128j..128j+127 full width: one fully
                        # contiguous 256KB DMA per j block
                        nc.sync.dma_start(out[js, :], ob[:])

                def outproj(st):
                    for jj in range(4):
                        outproj_j(4 * st + jj)

                def zip_emit(*thunk_lists):
                    """Round-robin emission of several streams of thunks."""
                    idx = [0] * len(thunk_lists)
                    while True:
                        progressed = False
                        for li, tl in enumerate(thunk_lists):
                            if idx[li] < len(tl):
                                t = tl[idx[li]]
                                if t is not None:
                                    t()
                                idx[li] += 1
                                progressed = True
                        if not progressed:
                            break

                def s_th(i, kcs):
                    return [(lambda kc=kc: em_s(i, [kc])) for kc in kcs]

                def p_th(i, kcs):
                    return [(lambda kc=kc: em_p(i, [kc])) for kc in kcs]

                def p_th2(i, kcs):
                    kcs = list(kcs)
                    return [
                        (lambda pair=kcs[k : k + 2]: em_p(i, pair))
                        for k in range(0, len(kcs), 2)
                    ]

                def steady(i, slist, ops=()):
                    """Steady-state block: PV+normalize of block i against the
                    score stream that feeds ScalarE two blocks ahead.  The PV
                    stream is padded so the first PV matmul of this block sits
                    ~6 score-emissions after the previous block's normalize --
                    the normalize chain (reciprocal etc.) takes ~6us and the
                    PE queue is in-order, so anything closer stalls the PE."""
                    sa = slist[0::2]
                    sb = slist[1::2]
                    pstream = [None] * 3 + p_th2(i, range(16)) + [lambda: em_f(i)]
                    opstream = [None] + list(ops)
                    zip_emit(pstream, sa, sb, opstream)

                # ====== emission ======
                # prologue: first scores as early as possible
                proj_qk_chain(0, 0)
                proj_qk_chain(0, 2)
                em_s(0, range(0, 4))
                proj_qk_chain(0, 1)
                proj_qk_chain(0, 3)
                em_s(1, range(0, 4))

                # paced interleave through the projection phase: proj chains
                # vs scores (feeding ScalarE) vs B0/B1 PV, with dependency
                # tracking so no thunk is emitted before its producers
                # v12-15 are deferred out of the PE-oversubscribed prologue
                # window; only late PV consumes them
                chains = (
                    [("q", 1, eb) for eb in (0, 2, 1, 3)]
                    + [("q", 2, eb) for eb in (0, 2, 1, 3)]
                    + [("v", j) for j in range(0, 4)]
                    + [("q", 3, eb) for eb in (0, 2, 1, 3)]
                    + [("v", j) for j in range(4, 12)]
                )
                score_q = []
                for lo in (4, 8, 12):
                    for kc in range(lo, lo + 4):
                        score_q.append((0, kc))
                        score_q.append((1, kc))
                for lo in (0, 4, 8):
                    for kc in range(lo, lo + 4):
                        score_q.append((2, kc))
                        score_q.append((3, kc))
                pv_q = (
                    [(0, kc) for kc in range(12)]
                    + [None] * 2
                    + [(1, kc) for kc in range(8)]
                )

                qdone = set()
                vdone = set()
                sdone = {(0, kc) for kc in range(4)} | {(1, kc) for kc in range(4)}

                def s_ready(it):
                    i, kc = it
                    cb = kc // 4
                    if cb >= 1 and not all((cb, eb) in qdone for eb in range(4)):
                        return False
                    if i >= 2 and not all((1, eb) in qdone for eb in range(4)):
                        return False
                    return True

                def p_ready(it):
                    if it is None:
                        return True
                    if it[0] == "F":
                        return True
                    i, kc = it
                    return (i, kc) in sdone and kc in vdone

                def p_do(it):
                    if it is None:
                        pass
                    elif it[0] == "F":
                        em_f(it[1])
                    else:
                        em_p(it[0], [it[1]])

                si = pi = 0
                for ci, ch in enumerate(chains):
                    if ch[0] == "q":
                        proj_qk_chain(ch[1], ch[2])
                        qdone.add((ch[1], ch[2]))
                    else:
                        proj_v_chain(ch[1])
                        vdone.add(ch[1])
                    starget = min(len(score_q), 2 * (ci + 1))
                    while si < starget and si < len(score_q) and s_ready(score_q[si]):
                        i, kc = score_q[si]
                        em_s(i, [kc])
                        sdone.add((i, kc))
                        si += 1
                    ptarget = min(len(pv_q), ci + 1)
                    while pi < ptarget and pi < len(pv_q) and p_ready(pv_q[pi]):
                        p_do(pv_q[pi])
                        pi += 1
                while si < len(score_q):
                    i, kc = score_q[si]
                    em_s(i, [kc])
                    sdone.add((i, kc))
                    si += 1
                    if si % 2 == 0 and pi < len(pv_q) and p_ready(pv_q[pi]):
                        p_do(pv_q[pi])
                        pi += 1
                while pi < len(pv_q):
                    p_do(pv_q[pi])
                    pi += 1

                for j in range(12, 16):
                    proj_v_chain(j)
                em_p(0, range(12, 16))
                em_f(0)
                em_p(1, range(8, 16))
                em_f(1)
                em_s(2, range(12, 16))
                em_s(3, range(12, 16))
                em_s(4, range(0, 8))
                # steady state: PV of block i round-robins with scores that
                # feed ScalarE two blocks ahead; outproj woven in per j-block
                op0 = [(lambda jj=jj: outproj_j(jj)) for jj in range(4)]
                steady(2, s_th(4, range(8, 16)) + s_th(5, range(0, 8)), op0)
                op1 = [(lambda jj=jj: outproj_j(4 + jj)) for jj in range(4)]
                steady(3, s_th(5, range(8, 16)) + s_th(6, range(0, 8)))
                steady(4, s_th(6, range(8, 16)) + s_th(7, range(0, 8)), op1)
                steady(5, s_th(7, range(8, 16)))
                # blocks 6 and 7 both chase their activates: B6 on pva/pvb
                # (free after F5's evacuation copies), B7 on the pj banks
                # (free after outproj(1)).  Both out-projections of st2/st3
                # run at the very end on the banks their finishes release.
                blocks[7]["ptags"] = ("pj", "pj", 2)
                zip_emit(p_th2(7, range(0, 16)) + [lambda: em_f(7)],
                         [None] * 2 + p_th2(6, range(16)) + [lambda: em_f(6)])
                for jj in range(4):
                    outproj_j(8 + jj, tags=("pva", "pvb"), tbufs=1)
                outproj(3)

    nc.compile()
    return nc


def make_in_maps(x, W_qkv, b_qkv, W_o):
    """Per-core input dicts (host-side sharding + layout prep).

    All weight/activation tensors are permuted to [128, ...] SBUF layout so
    on-device DMAs read fully contiguous dram rows."""
    x = np.asarray(x, np.float32)
    W_qkv = np.asarray(W_qkv, np.float32)
    b_qkv = np.asarray(b_qkv, np.float32)
    W_o = np.asarray(W_o, np.float32)

    def sb_layout(m):
        # [R rows, C cols] -> [128, (R//128) * C]: row 128*dc+p col c maps to
        # partition p, block dc, col c
        r, c = m.shape
        return np.ascontiguousarray(
            m.reshape(r // 128, 128, c).transpose(1, 0, 2).reshape(128, -1)
        ).astype(ml_dtypes.bfloat16)

    in_maps = []
    xt_blocks = []
    for b in range(B):
        xt = x[b].T  # [1024, 2048]
        xt_blocks.append(
            [sb_layout(np.ascontiguousarray(xt[:, 512 * cb : 512 * (cb + 1)])) for cb in range(4)]
        )
    for c in range(N_CORES):
        b, g = c // 4, c % 4
        heads = range(4 * g, 4 * g + 4)
        wq = [W_qkv[192 * h : 192 * h + 64] for h in heads]
        wk = [W_qkv[192 * h + 64 : 192 * h + 128] for h in heads]
        wv = [W_qkv[192 * h + 128 : 192 * h + 192] for h in heads]
        bq = [b_qkv[192 * h : 192 * h + 64] for h in heads]
        bk = [b_qkv[192 * h + 64 : 192 * h + 128] for h in heads]
        bv = [b_qkv[192 * h + 128 : 192 * h + 192] for h in heads]
        b_perm = np.concatenate(bq + bk + bv)  # [768]

        def wt(mats):
            # stack head blocks as rows then transpose to [1024, n]
            return sb_layout(np.ascontiguousarray(np.concatenate(mats, axis=0).T))

        im = {
            "w_a1": wt([wq[0], wq[1]]),
            "w_a2": wt([wk[0], wk[1]]),
            "w_b": wt([wq[2], wq[3], wk[2], wk[3]]),
            "w_v": wt(wv),
            "wo_t": sb_layout(
                np.ascontiguousarray(W_o[:, 256 * g : 256 * g + 256].T)
            ),
            "b_qk": np.ascontiguousarray(b_perm[:512].reshape(4, 128).T),
            "b_v": np.ascontiguousarray(np.broadcast_to(b_perm[512:], (128, 256))),
        }
        for cb in range(4):
            im[f"xt{cb}"] = xt_blocks[b][cb]
        in_maps.append(im)
    return in_maps


_NC = None


def kernel(x, W_qkv, b_qkv, W_o, b_o):
    global _NC
    from concourse.bass_utils import run_bass_kernel_spmd

    if _NC is None:
        _NC = build_kernel()
    in_maps = make_in_maps(x, W_qkv, b_qkv, W_o)
    res = run_bass_kernel_spmd(_NC, in_maps, core_ids=list(range(N_CORES)))
    b_o = np.asarray(b_o, np.float32)
    outs = [np.asarray(r["out"], np.float32) for r in res.results]
    full = np.empty((B, S, D_MODEL), np.float32)
    for b in range(B):
        full[b] = outs[4 * b] + outs[4 * b + 1] + outs[4 * b + 2] + outs[4 * b + 3]
        full[b] += b_o
    return full
